# revision 4
# baseline (speedup 1.0000x reference)
# Fused conv3x3(same) + bias + tanh + x2 + stride-4 subsample, data-parallel
# over 8 NeuronCores.
#
# Math: out[b,oc,y,x] = 2*tanh(sum_{ic,ky,kx} w[oc,ic,ky,kx]*x[b,ic,4y+ky-1,4x+kx-1] + bias[oc])
# computed in fp16 like the reference. Since the spatial stride (4) exceeds the
# kernel size (3), every output pixel reads a disjoint 3x3x8 input patch, so the
# conv lowers exactly to a [72 -> 64] GEMM over 64*64 pixels per image. The host
# does the im2col rearrangement (pure data movement, fp16 cast is identical to
# the reference's .astype(float16)); each core runs the GEMM + bias + tanh for
# 4 of the 32 images. The trailing *2 and fp32 cast are exact in either order,
# so they are applied on the host after the fp16 tanh.
import sys

import numpy as np

try:
    import concourse.bass as bass  # noqa: F401
except ImportError:
    sys.path.insert(0, "/opt/trn_rl_repo")

import concourse.bass as bass  # noqa: F401
import concourse.bacc as bacc
import concourse.mybir as mybir
from concourse.bass_utils import run_bass_kernel_spmd
from concourse.tile import TileContext

N_CORES = 8
B_FULL = 32
B_CORE = B_FULL // N_CORES  # 4 images per core
C_IN = 8
KH = KW = 3
K = C_IN * KH * KW  # 72 contraction
OC = 64
OH = OW = 64
NPIX = OH * OW  # 4096
F16 = mybir.dt.float16
F32 = mybir.dt.float32

_PROGRAM = None


def build_program():
    nc = bacc.Bacc("TRN2")
    xp = nc.dram_tensor("xp", [B_CORE, K, NPIX], F16, kind="ExternalInput")
    w = nc.dram_tensor("w", [K, OC], F16, kind="ExternalInput")
    bias = nc.dram_tensor("bias", [OC, 1], F16, kind="ExternalInput")
    y = nc.dram_tensor("y", [B_CORE, OC, NPIX], F16, kind="ExternalOutput")

    with TileContext(nc) as tc:
        with (
            tc.tile_pool(name="const", bufs=1) as cpool,
            tc.tile_pool(name="x", bufs=2) as xpool,
            tc.tile_pool(name="act", bufs=2) as apool,
            tc.tile_pool(name="psum", bufs=2, space="PSUM") as ppool,
        ):
            w_tile = cpool.tile([K, OC], F16)
            nc.sync.dma_start(out=w_tile[:], in_=w[:])
            b_tile = cpool.tile([OC, 1], F16)
            nc.sync.dma_start(out=b_tile[:], in_=bias[:])

            for b in range(B_CORE):
                x_tile = xpool.tile([K, NPIX], F16)
                nc.sync.dma_start(out=x_tile[:], in_=xp[b])
                act_tile = apool.tile([OC, NPIX], F16)
                for h in range(2):  # half image per PSUM tile (4 banks)
                    psum = ppool.tile([OC, 2048], F32)
                    for j in range(4):
                        col = h * 2048 + j * 512
                        nc.tensor.matmul(
                            psum[:, j * 512 : (j + 1) * 512],
                            w_tile[:],
                            x_tile[:, col : col + 512],
                            start=True,
                            stop=True,
                        )
                    nc.scalar.activation(
                        act_tile[:, h * 2048 : (h + 1) * 2048],
                        psum[:],
                        mybir.ActivationFunctionType.Tanh,
                        bias=b_tile[:],
                    )
                nc.sync.dma_start(out=y[b], in_=act_tile[:])
    nc.finalize()
    return nc


def _get_program():
    global _PROGRAM
    if _PROGRAM is None:
        _PROGRAM = build_program()
    return _PROGRAM


def _im2col(x: np.ndarray) -> np.ndarray:
    """[B,8,256,256] fp32 -> [B,72,4096] fp16 patches, p=(ky*3+kx)*8+ic."""
    B, C, H, W = x.shape
    xh = x.astype(np.float16)
    xpad = np.zeros((B, C, H + 2, W + 2), np.float16)
    xpad[:, :, 1 : H + 1, 1 : W + 1] = xh
    s = xpad.strides
    # windows[b,c,ky,kx,y,x] = xpad[b,c,4y+ky,4x+kx] = x[b,c,4y+ky-1,4x+kx-1]
    win = np.lib.stride_tricks.as_strided(
        xpad,
        shape=(B, C, KH, KW, OH, OW),
        strides=(s[0], s[1], s[2], s[3], 4 * s[2], 4 * s[3]),
    )
    return win.transpose(0, 2, 3, 1, 4, 5).reshape(B, K, NPIX)


def run_sharded(x, weight, bias, **spmd_kwargs):
    """Returns (output, BassKernelResults). spmd_kwargs e.g. trace=True."""
    patches = _im2col(x)  # [32, 72, 4096] f16, contiguous
    w_mat = np.ascontiguousarray(
        weight.transpose(2, 3, 1, 0).reshape(K, OC).astype(np.float16)
    )
    b_mat = np.ascontiguousarray(bias.astype(np.float16).reshape(OC, 1))

    in_maps = [
        {
            "xp": patches[c * B_CORE : (c + 1) * B_CORE],
            "w": w_mat,
            "bias": b_mat,
        }
        for c in range(N_CORES)
    ]
    nc = _get_program()
    res = run_bass_kernel_spmd(nc, in_maps, list(range(N_CORES)), **spmd_kwargs)
    y16 = np.concatenate([r["y"] for r in res.results], axis=0)  # [32,64,4096] f16
    # 2*tanh in fp16 then cast to fp32 == cast then *2 (exact: *2 is an
    # exponent bump, in-range for |tanh|<=1)
    out = y16.astype(np.float32).reshape(B_FULL, OC, OH, OW) * np.float32(2.0)
    return out, res


def kernel(x: np.ndarray, weight: np.ndarray, bias: np.ndarray) -> np.ndarray:
    return run_sharded(x, weight, bias)[0]


# revision 5
# speedup vs baseline: 1.1003x; 1.1003x over previous
# Fused conv3x3(same) + bias + tanh + x2 + stride-4 subsample, data-parallel
# over 8 NeuronCores.
#
# Math: out[b,oc,y,x] = 2*tanh(sum_{ic,ky,kx} w[oc,ic,ky,kx]*x[b,ic,4y+ky-1,4x+kx-1] + bias[oc])
# computed in fp16 like the reference. Since the spatial stride (4) exceeds the
# kernel size (3), every output pixel reads a disjoint 3x3x8 input patch, so the
# conv lowers exactly to a [72 -> 64] GEMM over 64*64 pixels per image. The host
# does the im2col rearrangement (pure data movement, fp16 cast is identical to
# the reference's .astype(float16)); each core runs the GEMM + bias + tanh for
# 4 of the 32 images. The trailing *2 and fp32 cast are exact in either order,
# so they are applied on the host after the fp16 tanh.
#
# Device layout: per image, the 8 N=512 matmul chunks are packed two-deep into
# PSUM partitions (chunk 2q -> partitions 0:64, chunk 2q+1 -> partitions 64:128
# of bank q) so one 128-partition ACT evaluates tanh for a whole image and the
# output DMA engages all SBUF ports. Output DRAM layout is [B, 2, 64, 2048]
# (t = chunk parity); the host interleaves it back.
import sys

import numpy as np

try:
    import concourse.bass as bass  # noqa: F401
except ImportError:
    sys.path.insert(0, "/opt/trn_rl_repo")

import concourse.bass as bass  # noqa: F401
import concourse.bacc as bacc
import concourse.mybir as mybir
from concourse.bass_utils import run_bass_kernel_spmd
from concourse.tile import TileContext

N_CORES = 8
B_FULL = 32
B_CORE = B_FULL // N_CORES  # 4 images per core
C_IN = 8
KH = KW = 3
K = C_IN * KH * KW  # 72 contraction
OC = 64
OH = OW = 64
NPIX = OH * OW  # 4096
F16 = mybir.dt.float16
F32 = mybir.dt.float32

_PROGRAM = None


def build_program():
    nc = bacc.Bacc("TRN2")
    xp = nc.dram_tensor("xp", [B_CORE, K, NPIX], F16, kind="ExternalInput")
    w = nc.dram_tensor("w", [K, OC], F16, kind="ExternalInput")
    bias = nc.dram_tensor("bias", [2 * OC, 1], F16, kind="ExternalInput")
    y = nc.dram_tensor("y", [B_CORE, 2, OC, NPIX // 2], F16, kind="ExternalOutput")

    with TileContext(nc) as tc:
        with (
            tc.tile_pool(name="const", bufs=1) as cpool,
            tc.tile_pool(name="x", bufs=3) as xpool,
            tc.tile_pool(name="act", bufs=3) as apool,
            tc.tile_pool(name="psum", bufs=2, space="PSUM") as ppool,
        ):
            w_tile = cpool.tile([K, OC], F16)
            nc.sync.dma_start(out=w_tile[:], in_=w[:])
            b_tile = cpool.tile([2 * OC, 1], F16)
            nc.sync.dma_start(out=b_tile[:], in_=bias[:])

            for b in range(B_CORE):
                x_tile = xpool.tile([K, NPIX], F16)
                nc.sync.dma_start(out=x_tile[:], in_=xp[b])
                # 8 chunks of 512 pixels, packed 2-deep in partitions x 4 banks
                psum = ppool.tile([2 * OC, NPIX // 2], F32)
                for j in range(8):
                    t, q = j % 2, j // 2
                    nc.tensor.matmul(
                        psum[t * OC : (t + 1) * OC, q * 512 : (q + 1) * 512],
                        w_tile[:],
                        x_tile[:, j * 512 : (j + 1) * 512],
                        start=True,
                        stop=True,
                    )
                act_tile = apool.tile([2 * OC, NPIX // 2], F16)
                nc.scalar.activation(
                    act_tile[:],
                    psum[:],
                    mybir.ActivationFunctionType.Tanh,
                    bias=b_tile[:],
                )
                # dest is contiguous per half; issued on the ACT HWDGE ring to
                # overlap with input loads on the SP ring
                nc.scalar.dma_start(out=y[b, 0], in_=act_tile[:OC, :])
                nc.scalar.dma_start(out=y[b, 1], in_=act_tile[OC:, :])
    nc.finalize()
    return nc


def _get_program():
    global _PROGRAM
    if _PROGRAM is None:
        _PROGRAM = build_program()
    return _PROGRAM


def _im2col(x: np.ndarray) -> np.ndarray:
    """[B,8,256,256] fp32 -> [B,72,4096] fp16 patches, p=(ky*3+kx)*8+ic."""
    B, C, H, W = x.shape
    xh = x.astype(np.float16)
    xpad = np.zeros((B, C, H + 2, W + 2), np.float16)
    xpad[:, :, 1 : H + 1, 1 : W + 1] = xh
    s = xpad.strides
    # windows[b,c,ky,kx,y,x] = xpad[b,c,4y+ky,4x+kx] = x[b,c,4y+ky-1,4x+kx-1]
    win = np.lib.stride_tricks.as_strided(
        xpad,
        shape=(B, C, KH, KW, OH, OW),
        strides=(s[0], s[1], s[2], s[3], 4 * s[2], 4 * s[3]),
    )
    return win.transpose(0, 2, 3, 1, 4, 5).reshape(B, K, NPIX)


def run_sharded(x, weight, bias, **spmd_kwargs):
    """Returns (output, BassKernelResults). spmd_kwargs e.g. trace=True."""
    patches = _im2col(x)  # [32, 72, 4096] f16, contiguous
    w_mat = np.ascontiguousarray(
        weight.transpose(2, 3, 1, 0).reshape(K, OC).astype(np.float16)
    )
    b_half = bias.astype(np.float16).reshape(OC, 1)
    b_mat = np.ascontiguousarray(np.concatenate([b_half, b_half], axis=0))

    in_maps = [
        {
            "xp": patches[c * B_CORE : (c + 1) * B_CORE],
            "w": w_mat,
            "bias": b_mat,
        }
        for c in range(N_CORES)
    ]
    nc = _get_program()
    res = run_bass_kernel_spmd(nc, in_maps, list(range(N_CORES)), **spmd_kwargs)
    # y core shard: [4, 2, 64, 2048]; chunk j = 512 pixels, j = 2q+t
    y16 = np.concatenate([r["y"] for r in res.results], axis=0)  # [32,2,64,2048]
    y16 = (
        y16.reshape(B_FULL, 2, OC, 4, 512)
        .transpose(0, 2, 3, 1, 4)
        .reshape(B_FULL, OC, NPIX)
    )
    # 2*tanh in fp16 then cast to fp32 == cast then *2 (exact: *2 is an
    # exponent bump, in-range for |tanh|<=1)
    out = y16.astype(np.float32).reshape(B_FULL, OC, OH, OW) * np.float32(2.0)
    return out, res


def kernel(x: np.ndarray, weight: np.ndarray, bias: np.ndarray) -> np.ndarray:
    return run_sharded(x, weight, bias)[0]
